# revision 1
# baseline (speedup 1.0000x reference)
"""Trainium2 Bass kernel for nn_AttentionModule (segment attention pooling).

Reference computation (N=2M nodes, D=64 feat, B=4096 graphs, batch sorted):
    seg_sum = segment_sum(x, batch)                  # [B, D]
    mean    = seg_sum / max(counts, 1)
    tg      = tanh(mean @ W)                         # [B, D]
    coef    = sigmoid(sum(x * tg[batch], -1))        # [N]
    out     = segment_sum(coef[:, None] * x, batch)  # [B, D]

Strategy: batch is sorted, so graphs are contiguous runs of rows.  Split the
4096 graphs into 8 groups of 512 (one per core) -> no cross-device reduction.
On the host, place every graph in a fixed-size slot of 128-node chunks
(zero-padded; zero rows are harmless in every stage) so the device program is
fully uniform across cores (SPMD).  Two slot classes cut padding: the
largest L5 graphs per core get CPL chunks, the rest CPS chunks.

The core processes 128-slot blocks end-to-end with the block's x resident in
SBUF (x is read from HBM exactly once):
  pass1: per chunk, PE matmul lhsT=x_chunk[128,64], rhs=ones[128,1]
         accumulates seg_sum^T [64, 128slots] in one PSUM bank.
  tg:    W-matmul on seg_sum^T, PE-transpose to rows, scale by 1/count
         (per-partition scalar), tanh on ACT, flatten rows to partition 0.
  pass2: per 16-slot group, two N=512 rank-1 matmuls broadcast the group's
         tg rows to all 128 partitions; per slot, DVE mul + reduce gives the
         per-node dots; sigmoid batched over 4 slots on ACT; scatter-add =
         matmul with lhsT = sigmoid column, rhs = x_chunk.
Blocks pipeline: block b+1's DMA/pass1 overlaps block b's pass2.
"""

import sys
import numpy as np

sys.path.insert(0, "/opt/trn_rl_repo")

import ml_dtypes  # noqa: E402
from contextlib import ExitStack  # noqa: E402

import concourse.bass as bass  # noqa: E402
import concourse.bacc as bacc  # noqa: E402
import concourse.tile as tile  # noqa: E402
from concourse import mybir  # noqa: E402
from concourse import bass_utils  # noqa: E402

P = 128          # partitions / nodes per chunk
D = 64           # feature dim
NCORES = 8
GRP = 16         # slots per tg-broadcast group
BLK = 128        # slots per pipelined unit
QUAD = 8         # slots per batched sigmoid
import os
TTR_MODE = int(os.environ.get("TTR_MODE", "0"))
BF16 = mybir.dt.bfloat16
F32 = mybir.dt.float32

_PROGRAM_CACHE: dict = {}


def _slot_layout(n_graphs, cpl, cps, l5):
    """chunk offsets per slot; slots [0,l5) have cpl chunks, rest cps."""
    cpgs = [cpl] * l5 + [cps] * (n_graphs - l5)
    offs = np.zeros(n_graphs + 1, dtype=np.int64)
    np.cumsum(cpgs, out=offs[1:])
    return cpgs, offs


def _build_program(n_graphs: int, cpl: int, cps: int, l5: int):
    nc = bacc.Bacc(
        "TRN2",
        target_bir_lowering=False,
        debug=False,
        enable_asserts=False,
        num_devices=NCORES,
    )
    cpgs, offs = _slot_layout(n_graphs, cpl, cps, l5)
    chunks = int(offs[-1])
    n_blk = n_graphs // BLK
    n_flush = n_graphs // 8

    xg = nc.dram_tensor("xg", [P, chunks * D], BF16, kind="ExternalInput")
    recip = nc.dram_tensor("recip", [BLK, n_blk], F32, kind="ExternalInput")
    wmat = nc.dram_tensor("wmat", [D, D], F32, kind="ExternalInput")
    out = nc.dram_tensor("out", [n_flush, 8 * D], F32, kind="ExternalOutput")

    with tile.TileContext(nc) as tc:
        with ExitStack() as ctx:
            consts = ctx.enter_context(tc.tile_pool(name="consts", bufs=1))
            small = ctx.enter_context(tc.tile_pool(name="small", bufs=1))
            xb_pool = ctx.enter_context(tc.tile_pool(name="xb", bufs=2))
            seg_ps_pool = ctx.enter_context(
                tc.tile_pool(name="segps", bufs=2, space="PSUM")
            )
            tg_ps_pool = ctx.enter_context(
                tc.tile_pool(name="tgps", bufs=1, space="PSUM")
            )
            tgb_ps_pool = ctx.enter_context(
                tc.tile_pool(name="tgbps", bufs=1, space="PSUM")
            )
            ops_pool = ctx.enter_context(tc.tile_pool(name="ops", bufs=2, space="PSUM"))
            seg_sb_pool = ctx.enter_context(tc.tile_pool(name="segsb", bufs=2))
            tgf_pool = ctx.enter_context(tc.tile_pool(name="tgf", bufs=1))
            tgb_sb_pool = ctx.enter_context(tc.tile_pool(name="tgbsb", bufs=3))
            cpool = ctx.enter_context(tc.tile_pool(name="coef", bufs=6))
            spool = ctx.enter_context(tc.tile_pool(name="scr", bufs=6))
            orow_pool = ctx.enter_context(tc.tile_pool(name="orow", bufs=2))

            ones_col = consts.tile([P, 1], BF16)
            nc.vector.memset(ones_col[:], 1.0)
            ones_row = consts.tile([1, P], BF16)
            nc.vector.memset(ones_row[:], 1.0)
            ones8 = consts.tile([P, 8 * 8], BF16)
            nc.vector.memset(ones8[:], 0.0)
            for j in range(8):
                nc.vector.memset(ones8[:, j * 8 + j:j * 8 + j + 1], 1.0)
            # identity for PE transpose: iota(f - p) == 0
            iota_pj = consts.tile([P, P], mybir.dt.int32)
            nc.gpsimd.iota(iota_pj[:], pattern=[[1, P]], base=0, channel_multiplier=-1)
            ident = consts.tile([P, P], F32)
            nc.vector.tensor_scalar(ident[:], iota_pj[:], 0, None, mybir.AluOpType.is_equal)

            w_sb = small.tile([D, D], F32)
            nc.sync.dma_start(w_sb[:], wmat[:])
            recip_sb = small.tile([BLK, n_blk], F32)
            nc.sync.dma_start(recip_sb[:], recip[:])

            for b in range(n_blk):
                s0 = b * BLK
                o0, o1 = int(offs[s0]), int(offs[s0 + BLK])
                cb = o1 - o0  # chunks in this block
                xb = xb_pool.tile([P, cb * D], BF16, tag="xb")
                nc.sync.dma_start(xb[:], xg[:, o0 * D:o1 * D])

                # ---- pass 1: seg row-sums, 8 slots per [8, 64] psum tile ----
                # tile-group t8 holds slots {j*8 + t8}; psum row j <-> slot j*8+t8,
                # so reassembly DMAs write contiguous partition ranges.
                segrows_sb = seg_sb_pool.tile([8, BLK // 8, D], F32, tag="segrows")
                for t8 in range(BLK // 8):
                    ps8 = seg_ps_pool.tile([8, D], F32, tag="ps8")
                    mms = []
                    for j in range(8):
                        s = s0 + j * (BLK // 8) + t8
                        cpg = cpgs[s]
                        xo = (int(offs[s]) - o0) * D
                        for k in range(cpg):
                            mms.append((j, xo + k * D))
                    for m, (j, xo) in enumerate(mms):
                        nc.tensor.matmul(
                            ps8[:],
                            ones8[:, j * 8:(j + 1) * 8],
                            xb[:, xo:xo + D],
                            start=(m == 0),
                            stop=(m == len(mms) - 1),
                        )
                    nc.scalar.copy(segrows_sb[:, t8, :], ps8[:])
                segrows_t = seg_sb_pool.tile([BLK, D], F32, tag="segrt")
                for j in range(8):
                    g8 = BLK // 8
                    nc.sync.dma_start(
                        segrows_t[j * g8:(j + 1) * g8, :], segrows_sb[j:j + 1, :, :]
                    )
                segT_ps = tg_ps_pool.tile([D, BLK], F32, tag="segTps")
                nc.tensor.transpose(segT_ps[:], segrows_t[:], ident[0:BLK, 0:BLK])
                segT_sb = seg_sb_pool.tile([D, BLK], F32, tag="segsb")
                nc.scalar.copy(segT_sb[:], segT_ps[:])

                # ---- tg rows -> tgflat [1, BLK*D] on partition 0 ----
                tgpre_ps = tg_ps_pool.tile([D, BLK], F32, tag="tgpre")
                nc.tensor.matmul(tgpre_ps[:], w_sb[:], segT_sb[:], start=True, stop=True)
                tgpre_sb = seg_sb_pool.tile([D, BLK], F32, tag="tgpresb")
                nc.scalar.copy(tgpre_sb[:], tgpre_ps[:])
                tp_ps = tg_ps_pool.tile([BLK, D], F32, tag="tp")
                nc.tensor.transpose(tp_ps[:], tgpre_sb[:], ident[0:D, 0:D])
                pre_sb = seg_sb_pool.tile([BLK, D], F32, tag="presb")
                nc.vector.tensor_scalar(
                    pre_sb[:], tp_ps[:], recip_sb[:, b:b + 1], None, mybir.AluOpType.mult
                )
                tgrows = seg_sb_pool.tile([BLK, D], BF16, tag="tgrows")
                nc.scalar.activation(
                    tgrows[:], pre_sb[:], mybir.ActivationFunctionType.Tanh
                )
                tgflat = tgf_pool.tile([1, BLK * D], BF16, tag="tgf")
                nc.sync.dma_start(tgflat[:], tgrows[:])

                # ---- pass 2 for the block ----
                for gi in range(BLK // GRP):
                    gs = s0 + gi * GRP
                    tgb_sb = tgb_sb_pool.tile([P, GRP * D], BF16, tag="tgbsb")
                    for h in range(2):
                        tgb_ps = tgb_ps_pool.tile([P, 512], F32, tag="tgbps")
                        nc.tensor.matmul(
                            tgb_ps[:],
                            ones_row[:],
                            tgflat[0:1, gi * GRP * D + h * 512:gi * GRP * D + (h + 1) * 512],
                            start=True,
                            stop=True,
                        )
                        nc.scalar.copy(tgb_sb[:, h * 512:(h + 1) * 512], tgb_ps[:])

                    for qi in range(GRP // QUAD):
                        qs = gs + qi * QUAD
                        qcpgs = [cpgs[qs + t] for t in range(QUAD)]
                        qoffs = np.concatenate([[0], np.cumsum(qcpgs)])
                        c4 = cpool.tile([P, int(qoffs[-1])], F32, tag="c4")
                        for t in range(QUAD):
                            s = qs + t
                            cpg = cpgs[s]
                            xo = (int(offs[s]) - o0) * D
                            if TTR_MODE:
                                scr = spool.tile([P, D], F32, tag="scrf")
                                for k in range(cpg):
                                    nc.vector.tensor_tensor_reduce(
                                        out=scr[:],
                                        in0=xb[:, xo + k * D:xo + (k + 1) * D],
                                        in1=tgb_sb[:, (s % GRP) * D:(s % GRP + 1) * D],
                                        scale=1.0,
                                        scalar=0.0,
                                        op0=mybir.AluOpType.mult,
                                        op1=mybir.AluOpType.add,
                                        accum_out=c4[:, int(qoffs[t]) + k:int(qoffs[t]) + k + 1],
                                    )
                            else:
                                scr = spool.tile([P, cpg * D], BF16, tag="scr")
                                tgb_rep = tgb_sb[
                                    :, (s % GRP) * D:(s % GRP + 1) * D
                                ].rearrange("p (k d) -> p k d", k=1).broadcast_to(
                                    [P, cpg, D]
                                )
                                nc.vector.tensor_tensor(
                                    scr[:], xb[:, xo:xo + cpg * D], tgb_rep,
                                    mybir.AluOpType.mult,
                                )
                                nc.vector.tensor_reduce(
                                    c4[:, int(qoffs[t]):int(qoffs[t + 1])],
                                    scr[:].rearrange("p (k d) -> p k d", k=cpg),
                                    mybir.AxisListType.X,
                                    mybir.AluOpType.add,
                                )
                        s4 = cpool.tile([P, int(qoffs[-1])], BF16, tag="s4")
                        nc.scalar.activation(
                            s4[:], c4[:], mybir.ActivationFunctionType.Sigmoid
                        )
                        for t in range(QUAD):
                            s = qs + t
                            cpg = cpgs[s]
                            xo = (int(offs[s]) - o0) * D
                            j = s % 8
                            if j == 0:
                                out_ps = ops_pool.tile([1, 8 * D], F32, tag="outps")
                            for k in range(cpg):
                                nc.tensor.matmul(
                                    out_ps[0:1, j * D:(j + 1) * D],
                                    s4[:, int(qoffs[t]) + k:int(qoffs[t]) + k + 1],
                                    xb[:, xo + k * D:xo + (k + 1) * D],
                                    start=(k == 0),
                                    stop=(k == cpg - 1),
                                )
                            if j == 7:
                                orow = orow_pool.tile([1, 8 * D], F32, tag="orow")
                                nc.scalar.copy(orow[:], out_ps[:])
                                nc.sync.dma_start(out[s // 8:s // 8 + 1, :], orow[:])

    nc.compile()
    return nc


def _layout_params(counts):
    """Uniform (cpl, cps, l5) across cores from the per-core count spread."""
    B = counts.shape[0]
    gpc = B // NCORES
    cpl = max(1, -(-int(counts.max()) // P))
    cps = min(4, cpl)
    l5 = 0
    for c in range(NCORES):
        l5 = max(l5, int((counts[c * gpc:(c + 1) * gpc] > cps * P).sum()))
    l5 = min(gpc, -(-l5 // GRP) * GRP)  # round up to group multiple
    if cpl == cps:
        l5 = 0
    return cpl, cps, l5


def _prep_inputs(x, batch, weight_matrix, size, cpl, cps, l5):
    """Host-side shard/sort/pad. Returns in_maps + per-core slot permutations."""
    B = int(size)
    N = x.shape[0]
    gpc = B // NCORES
    starts = np.searchsorted(batch, np.arange(B + 1)).astype(np.int64)
    counts = np.diff(starts)

    x_bf = np.ascontiguousarray(x, dtype=np.float32).astype(ml_dtypes.bfloat16)
    w32 = np.ascontiguousarray(weight_matrix, dtype=np.float32)

    cpgs, offs = _slot_layout(gpc, cpl, cps, l5)
    chunks = int(offs[-1])

    in_maps, perms = [], []
    for c in range(NCORES):
        glo, ghi = c * gpc, (c + 1) * gpc
        cnt = counts[glo:ghi]
        # biggest graphs into the L5 big slots (stable order otherwise)
        perm = np.argsort(-cnt, kind="stable")  # slot -> local graph
        if l5:
            assert cnt[perm[l5 - 1]] <= cpl * P and cnt[perm[l5:]].max(initial=0) <= cps * P
        else:
            assert cnt.max(initial=0) <= cps * P
        perms.append(perm)

        slot_of_graph = np.empty(gpc, dtype=np.int64)
        slot_of_graph[perm] = np.arange(gpc)
        nlo, nhi = starts[glo], starts[ghi]
        g_loc = np.asarray(batch[nlo:nhi], dtype=np.int64) - glo
        off_in_g = np.arange(nlo, nhi, dtype=np.int64) - starts[glo + g_loc]
        dest = offs[slot_of_graph[g_loc]] * P + off_in_g
        xpad = np.zeros((chunks * P, D), dtype=ml_dtypes.bfloat16)
        xpad[dest] = x_bf[nlo:nhi]
        xg_pm = np.ascontiguousarray(
            xpad.reshape(chunks, P, D).transpose(1, 0, 2).reshape(P, -1)
        )
        rc = 1.0 / np.maximum(cnt[perm].astype(np.float32), 1.0)
        recip_pm = np.ascontiguousarray(rc.reshape(gpc // 128, 128).T)
        in_maps.append({"xg": xg_pm, "recip": recip_pm, "wmat": w32})
    return in_maps, perms


def kernel(x, batch, weight_matrix, size, _return_results=False, _trace=False):
    x = np.asarray(x)
    batch = np.asarray(batch)
    weight_matrix = np.asarray(weight_matrix)
    B = int(size)
    assert B % (NCORES * P) == 0
    gpc = B // NCORES

    starts = np.searchsorted(batch, np.arange(B + 1))
    counts = np.diff(starts)
    cpl, cps, l5 = _layout_params(counts)

    key = (gpc, cpl, cps, l5)
    if key not in _PROGRAM_CACHE:
        _PROGRAM_CACHE[key] = _build_program(gpc, cpl, cps, l5)
    nc = _PROGRAM_CACHE[key]

    in_maps, perms = _prep_inputs(x, batch, weight_matrix, size, cpl, cps, l5)
    res = bass_utils.run_bass_kernel_spmd(
        nc, in_maps, core_ids=list(range(NCORES)), trace=_trace
    )
    full = np.empty((B, D), dtype=np.float32)
    for c in range(NCORES):
        o = res.results[c]["out"].reshape(gpc, D)  # slot-ordered
        full[c * gpc + perms[c]] = o
    if _return_results:
        return full, res
    return full



# revision 14
# speedup vs baseline: 1.2649x; 1.2649x over previous
"""Trainium2 Bass kernel for nn_AttentionModule (segment attention pooling).

Reference computation (N=2M nodes, D=64 feat, B=4096 graphs, batch sorted):
    seg_sum = segment_sum(x, batch)                  # [B, D]
    mean    = seg_sum / max(counts, 1)
    tg      = tanh(mean @ W)                         # [B, D]
    coef    = sigmoid(sum(x * tg[batch], -1))        # [N]
    out     = segment_sum(coef[:, None] * x, batch)  # [B, D]

batch is sorted, so graphs are contiguous runs of rows.  The 4096 graphs are
split into 8 groups of 512 (one per core) -> no cross-device reduction.  On
the host every graph goes into a fixed-size slot of 128-node chunks
(zero-padded; zero rows are harmless in every stage) so the device program is
fully uniform across cores (SPMD).  Two slot classes cut padding: the largest
L5 graphs per core get CPL chunks, the rest CPS chunks.

The core processes 64-slot blocks end-to-end with the block's x resident in
SBUF (x is read from HBM exactly once):
  pass1: per chunk, PE matmul lhsT=ones8 selector, rhs=x_chunk accumulates
         seg rows in one [8, 512] PSUM tile per block; single ACT drain.
  tg:    PE transpose -> W-matmul -> PE transpose; reciprocal-count scale is
         folded into the tanh activation's per-partition scale; tg rows are
         flattened to partition 0 and DMA-broadcast to all 128 partitions.
  pass2: per 16-slot group, ONE batched DVE multiply (bf16 2x mode) forms
         x*tg for all chunks, then an in-place pairwise-add tree (bf16 2x)
         reduces d=64 -> per-node dots; batched sigmoid on ACT; scatter-add =
         per-chunk matmul with lhsT = sigmoid column, slot-interleaved PSUM
         regions to avoid same-region accumulation stalls.
Blocks are software-pipelined: pass1/tg of block b is emitted before pass2 of
block b-1 so the PE stays busy while the DVE chews the previous block's dots.
"""

import os
import sys
import numpy as np

sys.path.insert(0, "/opt/trn_rl_repo")

import ml_dtypes  # noqa: E402
from contextlib import ExitStack  # noqa: E402

import concourse.bass as bass  # noqa: E402
import concourse.bacc as bacc  # noqa: E402
import concourse.tile as tile  # noqa: E402
from concourse import mybir  # noqa: E402
from concourse import bass_utils  # noqa: E402

P = 128          # partitions / nodes per chunk
D = 64           # feature dim
NCORES = 8
GRP = 16         # slots per dots batch (one DVE mult+tree per group)
BLK = 64         # slots per pipelined block
BF16 = mybir.dt.bfloat16
F32 = mybir.dt.float32

TGB_DMA = int(os.environ.get("TGB_DMA", "1"))  # 1: DMA broadcast, 0: PE matmul

_PROGRAM_CACHE: dict = {}


def _slot_layout(n_graphs, cpl, cps, l5):
    """chunk offsets per slot; slots [0,l5) have cpl chunks, rest cps."""
    cpgs = [cpl] * l5 + [cps] * (n_graphs - l5)
    offs = np.zeros(n_graphs + 1, dtype=np.int64)
    np.cumsum(cpgs, out=offs[1:])
    return cpgs, offs


def _build_program(n_graphs: int, cpl: int, cps: int, l5: int):
    nc = bacc.Bacc(
        "TRN2",
        target_bir_lowering=False,
        debug=False,
        enable_asserts=False,
        num_devices=NCORES,
    )
    cpgs, offs = _slot_layout(n_graphs, cpl, cps, l5)
    chunks = int(offs[-1])
    n_blk = n_graphs // BLK
    g8 = BLK // 8  # slots per psum row group (8)

    xg = nc.dram_tensor("xg", [P, chunks * D], BF16, kind="ExternalInput")
    recip = nc.dram_tensor("recip", [BLK, n_blk], F32, kind="ExternalInput")
    wmat = nc.dram_tensor("wmat", [D, D], F32, kind="ExternalInput")
    out = nc.dram_tensor("out", [n_graphs, D], F32, kind="ExternalOutput")

    with tile.TileContext(nc) as tc:
        with ExitStack() as ctx:
            consts = ctx.enter_context(tc.tile_pool(name="consts", bufs=1))
            small = ctx.enter_context(tc.tile_pool(name="small", bufs=1))
            xb_pool = ctx.enter_context(tc.tile_pool(name="xb", bufs=3))
            seg_ps_pool = ctx.enter_context(
                tc.tile_pool(name="segps", bufs=2, space="PSUM")
            )
            tg_ps_pool = ctx.enter_context(
                tc.tile_pool(name="tgps", bufs=1, space="PSUM")
            )
            ops_pool = ctx.enter_context(tc.tile_pool(name="ops", bufs=2, space="PSUM"))
            tgb_ps_pool = None
            if not TGB_DMA:
                tgb_ps_pool = ctx.enter_context(
                    tc.tile_pool(name="tgbps", bufs=1, space="PSUM")
                )
            seg_sb_pool = ctx.enter_context(tc.tile_pool(name="segsb", bufs=2))
            tgf_pool = ctx.enter_context(tc.tile_pool(name="tgf", bufs=2))
            tgb_pool = ctx.enter_context(tc.tile_pool(name="tgb", bufs=3))
            scr_pool = ctx.enter_context(tc.tile_pool(name="scr", bufs=2))
            c4_pool = ctx.enter_context(tc.tile_pool(name="c4", bufs=3))
            osb_pool = ctx.enter_context(tc.tile_pool(name="osb", bufs=2))

            ones8 = consts.tile([P, 8 * 8], BF16)
            nc.vector.memset(ones8[:], 0.0)
            for j in range(8):
                nc.vector.memset(ones8[:, j * 8 + j:j * 8 + j + 1], 1.0)
            ones_row = consts.tile([1, P], BF16)
            nc.vector.memset(ones_row[:], 1.0)
            # identity for PE transpose: iota(f - p) == 0
            iota_pj = consts.tile([P, P], mybir.dt.int32)
            nc.gpsimd.iota(iota_pj[:], pattern=[[1, P]], base=0, channel_multiplier=-1)
            ident = consts.tile([P, P], F32)
            nc.vector.tensor_scalar(ident[:], iota_pj[:], 0, None, mybir.AluOpType.is_equal)

            w_sb = small.tile([D, D], F32)
            nc.sync.dma_start(w_sb[:], wmat[:])
            recip_sb = small.tile([BLK, n_blk], F32)
            nc.sync.dma_start(recip_sb[:], recip[:])

            def front(b):
                """pass1 + tg chain for block b; returns (xb, tgflat)."""
                s0 = b * BLK
                o0, o1 = int(offs[s0]), int(offs[s0 + BLK])
                cb = o1 - o0
                xb = xb_pool.tile([P, cb * D], BF16, tag="xb")
                nc.sync.dma_start(xb[:], xg[:, o0 * D:o1 * D])

                # ---- pass 1: seg row sums into one [8, 512] psum tile ----
                # row group t8 holds slots {j*8 + t8}; psum row j, cols t8*64.
                seg_ps = seg_ps_pool.tile([8, 8 * D], F32, tag="segps")
                for t8 in range(g8):
                    mms = []
                    for j in range(8):
                        s = s0 + j * g8 + t8
                        xo = (int(offs[s]) - o0) * D
                        for k in range(cpgs[s]):
                            mms.append((j, xo + k * D))
                    for m, (j, xo) in enumerate(mms):
                        nc.tensor.matmul(
                            seg_ps[:, t8 * D:(t8 + 1) * D],
                            ones8[:, j * 8:(j + 1) * 8],
                            xb[:, xo:xo + D],
                            start=(m == 0),
                            stop=(m == len(mms) - 1),
                        )
                segrows_sb = seg_sb_pool.tile([8, g8, D], F32, tag="segrows")
                nc.scalar.copy(
                    segrows_sb[:], seg_ps[:].rearrange("p (t d) -> p t d", t=g8)
                )
                segrows_t = seg_sb_pool.tile([BLK, D], F32, tag="segrt")
                for j in range(8):
                    nc.sync.dma_start(
                        segrows_t[j * g8:(j + 1) * g8, :], segrows_sb[j:j + 1, :, :]
                    )
                # ---- tg: transpose -> W matmul -> transpose -> tanh ----
                segT_ps = tg_ps_pool.tile([D, BLK], F32, tag="segTps")
                nc.tensor.transpose(segT_ps[:], segrows_t[:], ident[0:BLK, 0:BLK])
                segT_sb = seg_sb_pool.tile([D, BLK], F32, tag="segsb")
                nc.scalar.copy(segT_sb[:], segT_ps[:])
                tgpre_ps = tg_ps_pool.tile([D, BLK], F32, tag="tgpre")
                nc.tensor.matmul(tgpre_ps[:], w_sb[:], segT_sb[:], start=True, stop=True)
                tgpre_sb = seg_sb_pool.tile([D, BLK], F32, tag="tgpresb")
                nc.scalar.copy(tgpre_sb[:], tgpre_ps[:])
                tp_ps = tg_ps_pool.tile([BLK, D], F32, tag="tp")
                nc.tensor.transpose(tp_ps[:], tgpre_sb[:], ident[0:D, 0:D])
                tgrows = seg_sb_pool.tile([BLK, D], BF16, tag="tgrows")
                nc.scalar.activation(
                    tgrows[:], tp_ps[:], mybir.ActivationFunctionType.Tanh,
                    scale=recip_sb[:, b:b + 1],
                )
                tgflat = tgf_pool.tile([1, BLK * D], BF16, tag="tgf")
                nc.sync.dma_start(tgflat[:], tgrows[:])
                return xb, tgflat

            def back(b, xb, tgflat):
                """dots + sigmoid + scatter for block b, drain to HBM."""
                s0 = b * BLK
                o0 = int(offs[s0])
                out_ps = ops_pool.tile([8, 8 * D], F32, tag="outps")
                for gi in range(BLK // GRP):
                    gs = s0 + gi * GRP
                    cpg = cpgs[gs]
                    xo = (int(offs[gs]) - o0) * D
                    ncols = GRP * cpg
                    # tg broadcast to all partitions
                    tgb = tgb_pool.tile([P, GRP * D], BF16, tag="tgb")
                    src = tgflat[0:1, gi * GRP * D:(gi + 1) * GRP * D]
                    if TGB_DMA:
                        nc.sync.dma_start(
                            tgb[:],
                            src.rearrange("p (q f) -> p q f", q=1)
                            .broadcast_to([1, P, GRP * D]),
                        )
                    else:
                        for h in range(GRP * D // 512):
                            tgb_ps = tgb_ps_pool.tile([P, 512], F32, tag="tgbps")
                            nc.tensor.matmul(
                                tgb_ps[:], ones_row[:],
                                src[:, h * 512:(h + 1) * 512],
                                start=True, stop=True,
                            )
                            nc.scalar.copy(tgb[:, h * 512:(h + 1) * 512], tgb_ps[:])
                    # batched multiply: scr[p, si, k, d] = x * tg[si]
                    scr = scr_pool.tile([P, ncols * D], BF16, tag="scr")
                    nc.vector.tensor_tensor(
                        scr[:].rearrange("p (s k d) -> p s k d", s=GRP, k=cpg),
                        xb[:, xo:xo + ncols * D]
                        .rearrange("p (s k d) -> p s k d", s=GRP, k=cpg),
                        tgb[:].rearrange("p (s q d) -> p s q d", s=GRP, q=1)
                        .broadcast_to([P, GRP, cpg, D]),
                        mybir.AluOpType.mult,
                    )
                    # in-place pairwise-add tree over d: 64 -> 1
                    scrv = scr[:].rearrange("p (c d) -> p c d", c=ncols)
                    w = D // 2
                    while w >= 1:
                        i0 = scrv[:, :, 0:w]
                        i1 = scrv[:, :, w:2 * w]
                        if w == 1:
                            c4 = c4_pool.tile([P, ncols], F32, tag="c4")
                            nc.vector.tensor_tensor(
                                c4[:].rearrange("p (c d) -> p c d", c=ncols),
                                i0, i1, mybir.AluOpType.add,
                            )
                        else:
                            nc.vector.tensor_tensor(i0, i0, i1, mybir.AluOpType.add)
                        w //= 2
                    # sigmoid written at stride 8 into a zero-padded tile so
                    # the scatter lhsT window s4z[:, q*8-j : q*8-j+8] has the
                    # coefficient at column j and zeros elsewhere (slot row is
                    # selected by lhsT column position, pass1-style M=8).
                    s4z = _get_s4z(cpg)
                    nc.scalar.activation(
                        s4z[:].rearrange("p (c e) -> p c e", c=ncols)[:, :, 0:1],
                        c4[:].rearrange("p (c e) -> p c e", c=ncols),
                        mybir.ActivationFunctionType.Sigmoid,
                    )
                    # scatter: slots g2*8+j of this group -> psum row j, col
                    # region (2gi+g2)*64; groups strictly sequential.
                    for g2 in range(GRP // 8):
                        reg = (2 * gi + g2) * D
                        mms = []
                        for j in range(8):
                            for k in range(cpg):
                                mms.append((j, (g2 * 8 + j) * cpg + k))
                        for m, (j, q) in enumerate(mms):
                            nc.tensor.matmul(
                                out_ps[:, reg:reg + D],
                                s4z[:, q * 8 - j:q * 8 - j + 8],
                                xb[:, xo + q * D:xo + (q + 1) * D],
                                start=(m == 0),
                                stop=(m == len(mms) - 1),
                            )
                osb = osb_pool.tile([8, g8, D], F32, tag="osb")
                nc.scalar.copy(
                    osb[:], out_ps[:].rearrange("p (g d) -> p g d", g=g8)
                )
                for j in range(8):
                    r0 = (b * 8 + j) * 8
                    nc.sync.dma_start(out[r0:r0 + 8, :], osb[j:j + 1, :, :])

            # persistent zero-padded sigmoid tiles, 3-way rotated per use so
            # the next group's sigmoid doesn't WAR-stall on the previous
            # group's scatter reads; zeros written once, only stride-8
            # positions are ever rewritten.
            s4z_tiles: dict = {}
            s4z_rot = [0]

            def _get_s4z(cpg):
                rot = s4z_rot[0] % 3
                s4z_rot[0] += 1
                key = (cpg, rot)
                if key not in s4z_tiles:
                    t = consts.tile([P, GRP * cpg * 8], BF16, tag=f"s4z{cpg}_{rot}")
                    nc.gpsimd.memset(t[:], 0.0)
                    s4z_tiles[key] = t
                return s4z_tiles[key]

            prev = None
            for b in range(n_blk):
                cur = front(b)
                if prev is not None:
                    back(b - 1, *prev)
                prev = cur
            back(n_blk - 1, *prev)

    nc.compile()
    return nc


def _layout_params(counts):
    """Uniform (cpl, cps, l5) across cores from the per-core count spread."""
    B = counts.shape[0]
    gpc = B // NCORES
    cpl = max(1, -(-int(counts.max()) // P))
    cps = min(4, cpl)
    l5 = 0
    for c in range(NCORES):
        l5 = max(l5, int((counts[c * gpc:(c + 1) * gpc] > cps * P).sum()))
    l5 = min(gpc, -(-l5 // GRP) * GRP)  # round up to group multiple
    if cpl == cps:
        l5 = 0
    return cpl, cps, l5


def _prep_inputs(x, batch, weight_matrix, size, cpl, cps, l5):
    """Host-side shard/sort/pad. Returns in_maps + per-core slot permutations."""
    B = int(size)
    gpc = B // NCORES
    starts = np.searchsorted(batch, np.arange(B + 1)).astype(np.int64)
    counts = np.diff(starts)

    x_bf = np.ascontiguousarray(x, dtype=np.float32).astype(ml_dtypes.bfloat16)
    w32 = np.ascontiguousarray(weight_matrix, dtype=np.float32)

    cpgs, offs = _slot_layout(gpc, cpl, cps, l5)
    chunks = int(offs[-1])

    in_maps, perms = [], []
    for c in range(NCORES):
        glo, ghi = c * gpc, (c + 1) * gpc
        cnt = counts[glo:ghi]
        # biggest graphs into the L5 big slots (stable order otherwise)
        perm = np.argsort(-cnt, kind="stable")  # slot -> local graph
        if l5:
            assert cnt[perm[l5 - 1]] <= cpl * P and cnt[perm[l5:]].max(initial=0) <= cps * P
        else:
            assert cnt.max(initial=0) <= cps * P
        perms.append(perm)

        slot_of_graph = np.empty(gpc, dtype=np.int64)
        slot_of_graph[perm] = np.arange(gpc)
        nlo, nhi = starts[glo], starts[ghi]
        g_loc = np.asarray(batch[nlo:nhi], dtype=np.int64) - glo
        off_in_g = np.arange(nlo, nhi, dtype=np.int64) - starts[glo + g_loc]
        dest = offs[slot_of_graph[g_loc]] * P + off_in_g
        xpad = np.zeros((chunks * P, D), dtype=ml_dtypes.bfloat16)
        xpad[dest] = x_bf[nlo:nhi]
        xg_pm = np.ascontiguousarray(
            xpad.reshape(chunks, P, D).transpose(1, 0, 2).reshape(P, -1)
        )
        rc = 1.0 / np.maximum(cnt[perm].astype(np.float32), 1.0)
        recip_pm = np.ascontiguousarray(rc.reshape(gpc // BLK, BLK).T)
        in_maps.append({"xg": xg_pm, "recip": recip_pm, "wmat": w32})
    return in_maps, perms


def _row_to_slot(gpc):
    """dram out row r = (b*8 + j)*8 + g  <->  block-local slot b*64 + g*8 + j."""
    r = np.arange(gpc)
    b, rem = r // BLK, r % BLK
    j, g = rem // 8, rem % 8
    return b * BLK + g * 8 + j


def kernel(x, batch, weight_matrix, size, _return_results=False, _trace=False):
    x = np.asarray(x)
    batch = np.asarray(batch)
    weight_matrix = np.asarray(weight_matrix)
    B = int(size)
    assert B % (NCORES * P) == 0
    gpc = B // NCORES

    starts = np.searchsorted(batch, np.arange(B + 1))
    counts = np.diff(starts)
    cpl, cps, l5 = _layout_params(counts)

    key = (gpc, cpl, cps, l5)
    if key not in _PROGRAM_CACHE:
        _PROGRAM_CACHE[key] = _build_program(gpc, cpl, cps, l5)
    nc = _PROGRAM_CACHE[key]

    in_maps, perms = _prep_inputs(x, batch, weight_matrix, size, cpl, cps, l5)
    res = bass_utils.run_bass_kernel_spmd(
        nc, in_maps, core_ids=list(range(NCORES)), trace=_trace
    )
    slot_of_row = _row_to_slot(gpc)
    full = np.empty((B, D), dtype=np.float32)
    for c in range(NCORES):
        o = res.results[c]["out"].reshape(gpc, D)  # row-ordered
        full[c * gpc + perms[c][slot_of_row]] = o
    if _return_results:
        return full, res
    return full


# revision 19
# speedup vs baseline: 1.2697x; 1.0037x over previous
"""Trainium2 Bass kernel for nn_AttentionModule (segment attention pooling).

Reference computation (N=2M nodes, D=64 feat, B=4096 graphs, batch sorted):
    seg_sum = segment_sum(x, batch)                  # [B, D]
    mean    = seg_sum / max(counts, 1)
    tg      = tanh(mean @ W)                         # [B, D]
    coef    = sigmoid(sum(x * tg[batch], -1))        # [N]
    out     = segment_sum(coef[:, None] * x, batch)  # [B, D]

batch is sorted, so graphs are contiguous runs of rows.  The 4096 graphs are
split into 8 groups of 512 (one per core) -> no cross-device reduction.  On
the host every graph goes into a fixed-size slot of 128-node chunks
(zero-padded; zero rows are harmless in every stage) so the device program is
fully uniform across cores (SPMD).  Two slot classes cut padding: the largest
L5 graphs per core get CPL chunks, the rest CPS chunks.

The core processes 64-slot blocks end-to-end with the block's x resident in
SBUF (x is read from HBM exactly once):
  pass1: per chunk, PE matmul lhsT=ones8 selector, rhs=x_chunk accumulates
         seg rows in one [8, 512] PSUM tile per block; single ACT drain.
  tg:    PE transpose -> W-matmul -> PE transpose; reciprocal-count scale is
         folded into the tanh activation's per-partition scale; tg rows are
         flattened to partition 0 and DMA-broadcast to all 128 partitions.
  pass2: per 16-slot group, ONE batched DVE multiply (bf16 2x mode) forms
         x*tg for all chunks, then an in-place pairwise-add tree (bf16 2x)
         reduces d=64 -> per-node dots; batched sigmoid on ACT; scatter-add =
         per-chunk matmul with lhsT = sigmoid column, slot-interleaved PSUM
         regions to avoid same-region accumulation stalls.
Blocks are software-pipelined: pass1/tg of block b is emitted before pass2 of
block b-1 so the PE stays busy while the DVE chews the previous block's dots.
"""

import os
import sys
import numpy as np

sys.path.insert(0, "/opt/trn_rl_repo")

import ml_dtypes  # noqa: E402
from contextlib import ExitStack  # noqa: E402

import concourse.bass as bass  # noqa: E402
import concourse.bacc as bacc  # noqa: E402
import concourse.tile as tile  # noqa: E402
from concourse import mybir  # noqa: E402
from concourse import bass_utils  # noqa: E402

P = 128          # partitions / nodes per chunk
D = 64           # feature dim
NCORES = 8
GRP = 16         # slots per dots batch (one DVE mult+tree per group)
BLK = 64         # slots per pipelined block
BF16 = mybir.dt.bfloat16
F32 = mybir.dt.float32

TGB_DMA = int(os.environ.get("TGB_DMA", "1"))  # 1: DMA broadcast, 0: PE matmul

_PROGRAM_CACHE: dict = {}


def _slot_layout(n_graphs, cpl, cps, l5):
    """chunk offsets per slot; slots [0,l5) have cpl chunks, rest cps."""
    cpgs = [cpl] * l5 + [cps] * (n_graphs - l5)
    offs = np.zeros(n_graphs + 1, dtype=np.int64)
    np.cumsum(cpgs, out=offs[1:])
    return cpgs, offs


def _build_program(n_graphs: int, cpl: int, cps: int, l5: int):
    nc = bacc.Bacc(
        "TRN2",
        target_bir_lowering=False,
        debug=False,
        enable_asserts=False,
        num_devices=NCORES,
    )
    cpgs, offs = _slot_layout(n_graphs, cpl, cps, l5)
    chunks = int(offs[-1])
    n_blk = n_graphs // BLK
    g8 = BLK // 8  # slots per psum row group (8)

    xg = nc.dram_tensor("xg", [P, chunks * D], BF16, kind="ExternalInput")
    recip = nc.dram_tensor("recip", [BLK, n_blk], F32, kind="ExternalInput")
    wmat = nc.dram_tensor("wmat", [D, D], F32, kind="ExternalInput")
    out = nc.dram_tensor("out", [n_graphs, D], F32, kind="ExternalOutput")

    with tile.TileContext(nc) as tc:
        with ExitStack() as ctx:
            consts = ctx.enter_context(tc.tile_pool(name="consts", bufs=1))
            small = ctx.enter_context(tc.tile_pool(name="small", bufs=1))
            xb_pool = ctx.enter_context(tc.tile_pool(name="xb", bufs=3))
            seg_ps_pool = ctx.enter_context(
                tc.tile_pool(name="segps", bufs=2, space="PSUM")
            )
            tg_ps_pool = ctx.enter_context(
                tc.tile_pool(name="tgps", bufs=1, space="PSUM")
            )
            ops_pool = ctx.enter_context(tc.tile_pool(name="ops", bufs=2, space="PSUM"))
            tgb_ps_pool = None
            if not TGB_DMA:
                tgb_ps_pool = ctx.enter_context(
                    tc.tile_pool(name="tgbps", bufs=1, space="PSUM")
                )
            seg_sb_pool = ctx.enter_context(tc.tile_pool(name="segsb", bufs=2))
            tgf_pool = ctx.enter_context(tc.tile_pool(name="tgf", bufs=2))
            tgb_pool = ctx.enter_context(tc.tile_pool(name="tgb", bufs=8))
            scr_pool = ctx.enter_context(tc.tile_pool(name="scr", bufs=2))
            c4_pool = ctx.enter_context(tc.tile_pool(name="c4", bufs=3))
            osb_pool = ctx.enter_context(tc.tile_pool(name="osb", bufs=2))

            ones8 = consts.tile([P, 8 * 8], BF16)
            nc.vector.memset(ones8[:], 0.0)
            for j in range(8):
                nc.vector.memset(ones8[:, j * 8 + j:j * 8 + j + 1], 1.0)
            ones_row = consts.tile([1, P], BF16)
            nc.vector.memset(ones_row[:], 1.0)
            # identity for PE transpose: iota(f - p) == 0
            iota_pj = consts.tile([P, P], mybir.dt.int32)
            nc.gpsimd.iota(iota_pj[:], pattern=[[1, P]], base=0, channel_multiplier=-1)
            ident = consts.tile([P, P], F32)
            nc.vector.tensor_scalar(ident[:], iota_pj[:], 0, None, mybir.AluOpType.is_equal)

            w_sb = small.tile([D, D], F32)
            nc.sync.dma_start(w_sb[:], wmat[:])
            recip_sb = small.tile([BLK, n_blk], F32)
            nc.sync.dma_start(recip_sb[:], recip[:])

            def front(b):
                """pass1 + tg chain for block b; returns (xb, tgflat)."""
                s0 = b * BLK
                o0, o1 = int(offs[s0]), int(offs[s0 + BLK])
                cb = o1 - o0
                xb = xb_pool.tile([P, cb * D], BF16, tag="xb")
                nc.sync.dma_start(xb[:], xg[:, o0 * D:o1 * D])

                # ---- pass 1: seg row sums into one [8, 512] psum tile ----
                # row group t8 holds slots {j*8 + t8}; psum row j, cols t8*64.
                seg_ps = seg_ps_pool.tile([8, 8 * D], F32, tag="segps")
                for t8 in range(g8):
                    mms = []
                    for j in range(8):
                        s = s0 + j * g8 + t8
                        xo = (int(offs[s]) - o0) * D
                        for k in range(cpgs[s]):
                            mms.append((j, xo + k * D))
                    for m, (j, xo) in enumerate(mms):
                        nc.tensor.matmul(
                            seg_ps[:, t8 * D:(t8 + 1) * D],
                            ones8[:, j * 8:(j + 1) * 8],
                            xb[:, xo:xo + D],
                            start=(m == 0),
                            stop=(m == len(mms) - 1),
                        )
                segrows_sb = seg_sb_pool.tile([8, g8, D], F32, tag="segrows")
                nc.scalar.copy(
                    segrows_sb[:], seg_ps[:].rearrange("p (t d) -> p t d", t=g8)
                )
                segrows_t = seg_sb_pool.tile([BLK, D], F32, tag="segrt")
                for j in range(8):
                    nc.sync.dma_start(
                        segrows_t[j * g8:(j + 1) * g8, :], segrows_sb[j:j + 1, :, :]
                    )
                # ---- tg: transpose -> W matmul -> transpose -> tanh ----
                segT_ps = tg_ps_pool.tile([D, BLK], F32, tag="segTps")
                nc.tensor.transpose(segT_ps[:], segrows_t[:], ident[0:BLK, 0:BLK])
                segT_sb = seg_sb_pool.tile([D, BLK], F32, tag="segsb")
                nc.scalar.copy(segT_sb[:], segT_ps[:])
                tgpre_ps = tg_ps_pool.tile([D, BLK], F32, tag="tgpre")
                nc.tensor.matmul(tgpre_ps[:], w_sb[:], segT_sb[:], start=True, stop=True)
                tgpre_sb = seg_sb_pool.tile([D, BLK], F32, tag="tgpresb")
                nc.scalar.copy(tgpre_sb[:], tgpre_ps[:])
                tp_ps = tg_ps_pool.tile([BLK, D], F32, tag="tp")
                nc.tensor.transpose(tp_ps[:], tgpre_sb[:], ident[0:D, 0:D])
                tgrows = seg_sb_pool.tile([BLK, D], BF16, tag="tgrows")
                nc.scalar.activation(
                    tgrows[:], tp_ps[:], mybir.ActivationFunctionType.Tanh,
                    scale=recip_sb[:, b:b + 1],
                )
                tgflat = tgf_pool.tile([1, BLK * D], BF16, tag="tgf")
                nc.sync.dma_start(tgflat[:], tgrows[:])
                return xb, tgflat

            def tgb_bcast(b, tgflat):
                """Broadcast block b's tg rows to all 128 partitions, one
                [P, GRP*D] tile per group.  Emitted a full block ahead of
                back(b) so the DVE multiply never waits on PE progress."""
                tgbs = []
                for gi in range(BLK // GRP):
                    tgb = tgb_pool.tile([P, GRP * D], BF16, tag="tgb")
                    src = tgflat[0:1, gi * GRP * D:(gi + 1) * GRP * D]
                    if TGB_DMA:
                        nc.sync.dma_start(
                            tgb[:],
                            src.rearrange("p (q f) -> p q f", q=1)
                            .broadcast_to([1, P, GRP * D]),
                        )
                    else:
                        for h in range(GRP * D // 512):
                            tgb_ps = tgb_ps_pool.tile([P, 512], F32, tag="tgbps")
                            nc.tensor.matmul(
                                tgb_ps[:], ones_row[:],
                                src[:, h * 512:(h + 1) * 512],
                                start=True, stop=True,
                            )
                            nc.scalar.copy(tgb[:, h * 512:(h + 1) * 512], tgb_ps[:])
                    tgbs.append(tgb)
                return tgbs

            def back(b, xb, tgbs):
                """dots + sigmoid + scatter for block b, drain to HBM."""
                s0 = b * BLK
                o0 = int(offs[s0])
                out_ps = ops_pool.tile([8, 8 * D], F32, tag="outps")
                for gi in range(BLK // GRP):
                    gs = s0 + gi * GRP
                    cpg = cpgs[gs]
                    xo = (int(offs[gs]) - o0) * D
                    ncols = GRP * cpg
                    tgb = tgbs[gi]
                    # batched multiply: scr[p, si, k, d] = x * tg[si]
                    scr = scr_pool.tile([P, ncols * D], BF16, tag="scr")
                    nc.vector.tensor_tensor(
                        scr[:].rearrange("p (s k d) -> p s k d", s=GRP, k=cpg),
                        xb[:, xo:xo + ncols * D]
                        .rearrange("p (s k d) -> p s k d", s=GRP, k=cpg),
                        tgb[:].rearrange("p (s q d) -> p s q d", s=GRP, q=1)
                        .broadcast_to([P, GRP, cpg, D]),
                        mybir.AluOpType.mult,
                    )
                    # in-place pairwise-add tree over d: 64 -> 1
                    scrv = scr[:].rearrange("p (c d) -> p c d", c=ncols)
                    w = D // 2
                    while w >= 1:
                        i0 = scrv[:, :, 0:w]
                        i1 = scrv[:, :, w:2 * w]
                        if w == 1:
                            c4 = c4_pool.tile([P, ncols], F32, tag="c4")
                            nc.vector.tensor_tensor(
                                c4[:].rearrange("p (c d) -> p c d", c=ncols),
                                i0, i1, mybir.AluOpType.add,
                            )
                        else:
                            nc.vector.tensor_tensor(i0, i0, i1, mybir.AluOpType.add)
                        w //= 2
                    # sigmoid written at stride 8 into a zero-padded tile so
                    # the scatter lhsT window s4z[:, q*8-j : q*8-j+8] has the
                    # coefficient at column j and zeros elsewhere (slot row is
                    # selected by lhsT column position, pass1-style M=8).
                    s4z = _get_s4z(cpg)
                    nc.scalar.activation(
                        s4z[:].rearrange("p (c e) -> p c e", c=ncols)[:, :, 0:1],
                        c4[:].rearrange("p (c e) -> p c e", c=ncols),
                        mybir.ActivationFunctionType.Sigmoid,
                    )
                    # scatter: slots g2*8+j of this group -> psum row j, col
                    # region (2gi+g2)*64; groups strictly sequential.
                    for g2 in range(GRP // 8):
                        reg = (2 * gi + g2) * D
                        mms = []
                        for j in range(8):
                            for k in range(cpg):
                                mms.append((j, (g2 * 8 + j) * cpg + k))
                        for m, (j, q) in enumerate(mms):
                            nc.tensor.matmul(
                                out_ps[:, reg:reg + D],
                                s4z[:, q * 8 - j:q * 8 - j + 8],
                                xb[:, xo + q * D:xo + (q + 1) * D],
                                start=(m == 0),
                                stop=(m == len(mms) - 1),
                            )
                osb = osb_pool.tile([8, g8, D], F32, tag="osb")
                nc.scalar.copy(
                    osb[:], out_ps[:].rearrange("p (g d) -> p g d", g=g8)
                )
                for j in range(8):
                    r0 = (b * 8 + j) * 8
                    nc.sync.dma_start(out[r0:r0 + 8, :], osb[j:j + 1, :, :])

            # persistent zero-padded sigmoid tiles, 3-way rotated per use so
            # the next group's sigmoid doesn't WAR-stall on the previous
            # group's scatter reads; zeros written once, only stride-8
            # positions are ever rewritten.
            s4z_tiles: dict = {}
            s4z_rot = [0]

            def _get_s4z(cpg):
                rot = s4z_rot[0] % 3
                s4z_rot[0] += 1
                key = (cpg, rot)
                if key not in s4z_tiles:
                    t = consts.tile([P, GRP * cpg * 8], BF16, tag=f"s4z{cpg}_{rot}")
                    nc.gpsimd.memset(t[:], 0.0)
                    s4z_tiles[key] = t
                return s4z_tiles[key]

            prev = None
            for b in range(n_blk):
                xb_t, tgflat_t = front(b)
                if prev is not None:
                    back(b - 1, prev[0], prev[1])
                tgbs_t = tgb_bcast(b, tgflat_t)
                prev = (xb_t, tgbs_t)
            back(n_blk - 1, prev[0], prev[1])

    nc.compile()
    return nc


def _layout_params(counts):
    """Uniform (cpl, cps, l5) across cores from the per-core count spread."""
    B = counts.shape[0]
    gpc = B // NCORES
    cpl = max(1, -(-int(counts.max()) // P))
    cps = min(4, cpl)
    l5 = 0
    for c in range(NCORES):
        l5 = max(l5, int((counts[c * gpc:(c + 1) * gpc] > cps * P).sum()))
    l5 = min(gpc, -(-l5 // GRP) * GRP)  # round up to group multiple
    if cpl == cps:
        l5 = 0
    return cpl, cps, l5


def _prep_inputs(x, batch, weight_matrix, size, cpl, cps, l5):
    """Host-side shard/sort/pad. Returns in_maps + per-core slot permutations."""
    B = int(size)
    gpc = B // NCORES
    starts = np.searchsorted(batch, np.arange(B + 1)).astype(np.int64)
    counts = np.diff(starts)

    x_bf = np.ascontiguousarray(x, dtype=np.float32).astype(ml_dtypes.bfloat16)
    w32 = np.ascontiguousarray(weight_matrix, dtype=np.float32)

    cpgs, offs = _slot_layout(gpc, cpl, cps, l5)
    chunks = int(offs[-1])

    in_maps, perms = [], []
    for c in range(NCORES):
        glo, ghi = c * gpc, (c + 1) * gpc
        cnt = counts[glo:ghi]
        # biggest graphs into the L5 big slots (stable order otherwise)
        perm = np.argsort(-cnt, kind="stable")  # slot -> local graph
        if l5:
            assert cnt[perm[l5 - 1]] <= cpl * P and cnt[perm[l5:]].max(initial=0) <= cps * P
        else:
            assert cnt.max(initial=0) <= cps * P
        perms.append(perm)

        slot_of_graph = np.empty(gpc, dtype=np.int64)
        slot_of_graph[perm] = np.arange(gpc)
        nlo, nhi = starts[glo], starts[ghi]
        g_loc = np.asarray(batch[nlo:nhi], dtype=np.int64) - glo
        off_in_g = np.arange(nlo, nhi, dtype=np.int64) - starts[glo + g_loc]
        dest = offs[slot_of_graph[g_loc]] * P + off_in_g
        xpad = np.zeros((chunks * P, D), dtype=ml_dtypes.bfloat16)
        xpad[dest] = x_bf[nlo:nhi]
        xg_pm = np.ascontiguousarray(
            xpad.reshape(chunks, P, D).transpose(1, 0, 2).reshape(P, -1)
        )
        rc = 1.0 / np.maximum(cnt[perm].astype(np.float32), 1.0)
        recip_pm = np.ascontiguousarray(rc.reshape(gpc // BLK, BLK).T)
        in_maps.append({"xg": xg_pm, "recip": recip_pm, "wmat": w32})
    return in_maps, perms


def _row_to_slot(gpc):
    """dram out row r = (b*8 + j)*8 + g  <->  block-local slot b*64 + g*8 + j."""
    r = np.arange(gpc)
    b, rem = r // BLK, r % BLK
    j, g = rem // 8, rem % 8
    return b * BLK + g * 8 + j


def kernel(x, batch, weight_matrix, size, _return_results=False, _trace=False):
    x = np.asarray(x)
    batch = np.asarray(batch)
    weight_matrix = np.asarray(weight_matrix)
    B = int(size)
    assert B % (NCORES * P) == 0
    gpc = B // NCORES

    starts = np.searchsorted(batch, np.arange(B + 1))
    counts = np.diff(starts)
    cpl, cps, l5 = _layout_params(counts)

    key = (gpc, cpl, cps, l5)
    if key not in _PROGRAM_CACHE:
        _PROGRAM_CACHE[key] = _build_program(gpc, cpl, cps, l5)
    nc = _PROGRAM_CACHE[key]

    in_maps, perms = _prep_inputs(x, batch, weight_matrix, size, cpl, cps, l5)
    res = bass_utils.run_bass_kernel_spmd(
        nc, in_maps, core_ids=list(range(NCORES)), trace=_trace
    )
    slot_of_row = _row_to_slot(gpc)
    full = np.empty((B, D), dtype=np.float32)
    for c in range(NCORES):
        o = res.results[c]["out"].reshape(gpc, D)  # row-ordered
        full[c * gpc + perms[c][slot_of_row]] = o
    if _return_results:
        return full, res
    return full


# revision 22
# speedup vs baseline: 1.2960x; 1.0208x over previous
"""Trainium2 Bass kernel for nn_AttentionModule (segment attention pooling).

Reference computation (N=2M nodes, D=64 feat, B=4096 graphs, batch sorted):
    seg_sum = segment_sum(x, batch)                  # [B, D]
    mean    = seg_sum / max(counts, 1)
    tg      = tanh(mean @ W)                         # [B, D]
    coef    = sigmoid(sum(x * tg[batch], -1))        # [N]
    out     = segment_sum(coef[:, None] * x, batch)  # [B, D]

batch is sorted, so graphs are contiguous runs of rows.  The 4096 graphs are
split into 8 groups of 512 (one per core) -> no cross-device reduction.  On
the host every graph goes into a fixed-size slot of 128-node chunks
(zero-padded; zero rows are harmless in every stage) so the device program is
fully uniform across cores (SPMD).  Two slot classes cut padding: the largest
L5 graphs per core get CPL chunks, the rest CPS chunks.

The core processes 64-slot blocks end-to-end with the block's x resident in
SBUF (x is read from HBM exactly once):
  pass1: per chunk, PE matmul lhsT=ones8 selector, rhs=x_chunk accumulates
         seg rows in one [8, 512] PSUM tile per block; single ACT drain.
  tg:    PE transpose -> W-matmul -> PE transpose; reciprocal-count scale is
         folded into the tanh activation's per-partition scale; tg rows are
         flattened to partition 0 and DMA-broadcast to all 128 partitions.
  pass2: per 16-slot group, ONE batched DVE multiply (bf16 2x mode) forms
         x*tg for all chunks, then an in-place pairwise-add tree (bf16 2x)
         reduces d=64 -> per-node dots; batched sigmoid on ACT; scatter-add =
         per-chunk matmul with lhsT = sigmoid column, slot-interleaved PSUM
         regions to avoid same-region accumulation stalls.
Blocks are software-pipelined: pass1/tg of block b is emitted before pass2 of
block b-1 so the PE stays busy while the DVE chews the previous block's dots.
"""

import os
import sys
import numpy as np

sys.path.insert(0, "/opt/trn_rl_repo")

import ml_dtypes  # noqa: E402
from contextlib import ExitStack  # noqa: E402

import concourse.bass as bass  # noqa: E402
import concourse.bacc as bacc  # noqa: E402
import concourse.tile as tile  # noqa: E402
from concourse import mybir  # noqa: E402
from concourse import bass_utils  # noqa: E402

P = 128          # partitions / nodes per chunk
D = 64           # feature dim
NCORES = 8
GRP = 16         # slots per dots batch (one DVE mult+tree per group)
BLK = 64         # slots per pipelined block
BF16 = mybir.dt.bfloat16
F32 = mybir.dt.float32

TGB_DMA = int(os.environ.get("TGB_DMA", "1"))  # 1: DMA broadcast, 0: PE matmul

_PROGRAM_CACHE: dict = {}


def _slot_layout(n_graphs, cpl, cps, l5):
    """chunk offsets per slot; slots [0,l5) have cpl chunks, rest cps."""
    cpgs = [cpl] * l5 + [cps] * (n_graphs - l5)
    offs = np.zeros(n_graphs + 1, dtype=np.int64)
    np.cumsum(cpgs, out=offs[1:])
    return cpgs, offs


def _build_program(n_graphs: int, cpl: int, cps: int, l5: int):
    nc = bacc.Bacc(
        "TRN2",
        target_bir_lowering=False,
        debug=False,
        enable_asserts=False,
        num_devices=NCORES,
    )
    cpgs, offs = _slot_layout(n_graphs, cpl, cps, l5)
    chunks = int(offs[-1])
    n_blk = n_graphs // BLK
    g8 = BLK // 8  # slots per psum row group (8)

    xg = nc.dram_tensor("xg", [P, chunks * D], BF16, kind="ExternalInput")
    recip = nc.dram_tensor("recip", [BLK, n_blk], F32, kind="ExternalInput")
    wmat = nc.dram_tensor("wmat", [D, D], F32, kind="ExternalInput")
    out = nc.dram_tensor("out", [n_graphs, D], F32, kind="ExternalOutput")

    with tile.TileContext(nc) as tc:
        with ExitStack() as ctx:
            consts = ctx.enter_context(tc.tile_pool(name="consts", bufs=1))
            small = ctx.enter_context(tc.tile_pool(name="small", bufs=1))
            xb_pool = ctx.enter_context(tc.tile_pool(name="xb", bufs=3))
            seg_ps_pool = ctx.enter_context(
                tc.tile_pool(name="segps", bufs=2, space="PSUM")
            )
            tg_ps_pool = ctx.enter_context(
                tc.tile_pool(name="tgps", bufs=1, space="PSUM")
            )
            ops_pool = ctx.enter_context(tc.tile_pool(name="ops", bufs=2, space="PSUM"))
            tgb_ps_pool = None
            if not TGB_DMA:
                tgb_ps_pool = ctx.enter_context(
                    tc.tile_pool(name="tgbps", bufs=1, space="PSUM")
                )
            seg_sb_pool = ctx.enter_context(tc.tile_pool(name="segsb", bufs=2))
            tgf_pool = ctx.enter_context(tc.tile_pool(name="tgf", bufs=2))
            tgb_pool = ctx.enter_context(tc.tile_pool(name="tgb", bufs=8))
            scr_pool = ctx.enter_context(tc.tile_pool(name="scr", bufs=2))
            c4_pool = ctx.enter_context(tc.tile_pool(name="c4", bufs=3))
            osb_pool = ctx.enter_context(tc.tile_pool(name="osb", bufs=2))

            ones8 = consts.tile([P, 8 * 8], BF16)
            nc.vector.memset(ones8[:], 0.0)
            for j in range(8):
                nc.vector.memset(ones8[:, j * 8 + j:j * 8 + j + 1], 1.0)
            ones_row = consts.tile([1, P], BF16)
            nc.vector.memset(ones_row[:], 1.0)
            # identity for PE transpose: iota(f - p) == 0
            iota_pj = consts.tile([P, P], mybir.dt.int32)
            nc.gpsimd.iota(iota_pj[:], pattern=[[1, P]], base=0, channel_multiplier=-1)
            ident = consts.tile([P, P], F32)
            nc.vector.tensor_scalar(ident[:], iota_pj[:], 0, None, mybir.AluOpType.is_equal)

            w_sb = small.tile([D, D], F32)
            nc.sync.dma_start(w_sb[:], wmat[:])
            recip_sb = small.tile([BLK, n_blk], F32)
            nc.sync.dma_start(recip_sb[:], recip[:])

            def front(b):
                """pass1 + tg chain for block b; returns (xb, tgflat)."""
                s0 = b * BLK
                o0, o1 = int(offs[s0]), int(offs[s0 + BLK])
                cb = o1 - o0
                xb = xb_pool.tile([P, cb * D], BF16, tag="xb")
                nc.sync.dma_start(xb[:], xg[:, o0 * D:o1 * D])

                # ---- pass 1: seg row sums into one [8, 512] psum tile ----
                # row group t8 holds slots {j*8 + t8}; psum row j, cols t8*64.
                seg_ps = seg_ps_pool.tile([8, 8 * D], F32, tag="segps")
                for t8 in range(g8):
                    mms = []
                    for j in range(8):
                        s = s0 + j * g8 + t8
                        xo = (int(offs[s]) - o0) * D
                        for k in range(cpgs[s]):
                            mms.append((j, xo + k * D))
                    for m, (j, xo) in enumerate(mms):
                        nc.tensor.matmul(
                            seg_ps[:, t8 * D:(t8 + 1) * D],
                            ones8[:, j * 8:(j + 1) * 8],
                            xb[:, xo:xo + D],
                            start=(m == 0),
                            stop=(m == len(mms) - 1),
                        )
                segrows_sb = seg_sb_pool.tile([8, g8, D], F32, tag="segrows")
                nc.scalar.copy(
                    segrows_sb[:], seg_ps[:].rearrange("p (t d) -> p t d", t=g8)
                )
                segrows_t = seg_sb_pool.tile([BLK, D], F32, tag="segrt")
                for j in range(8):
                    nc.sync.dma_start(
                        segrows_t[j * g8:(j + 1) * g8, :], segrows_sb[j:j + 1, :, :]
                    )
                # ---- tg: transpose -> W matmul -> transpose -> tanh ----
                segT_ps = tg_ps_pool.tile([D, BLK], F32, tag="segTps")
                nc.tensor.transpose(segT_ps[:], segrows_t[:], ident[0:BLK, 0:BLK])
                segT_sb = seg_sb_pool.tile([D, BLK], F32, tag="segsb")
                nc.scalar.copy(segT_sb[:], segT_ps[:])
                tgpre_ps = tg_ps_pool.tile([D, BLK], F32, tag="tgpre")
                nc.tensor.matmul(tgpre_ps[:], w_sb[:], segT_sb[:], start=True, stop=True)
                tgpre_sb = seg_sb_pool.tile([D, BLK], F32, tag="tgpresb")
                nc.scalar.copy(tgpre_sb[:], tgpre_ps[:])
                tp_ps = tg_ps_pool.tile([BLK, D], F32, tag="tp")
                nc.tensor.transpose(tp_ps[:], tgpre_sb[:], ident[0:D, 0:D])
                tgrows = seg_sb_pool.tile([BLK, D], BF16, tag="tgrows")
                nc.scalar.activation(
                    tgrows[:], tp_ps[:], mybir.ActivationFunctionType.Tanh,
                    scale=recip_sb[:, b:b + 1],
                )
                tgflat = tgf_pool.tile([1, BLK * D], BF16, tag="tgf")
                nc.sync.dma_start(tgflat[:], tgrows[:])
                return xb, tgflat

            def tgb_one(tgflat, gi):
                """Broadcast one group's tg rows to all 128 partitions."""
                tgb = tgb_pool.tile([P, GRP * D], BF16, tag="tgb")
                src = tgflat[0:1, gi * GRP * D:(gi + 1) * GRP * D]
                if TGB_DMA:
                    nc.sync.dma_start(
                        tgb[:],
                        src.rearrange("p (q f) -> p q f", q=1)
                        .broadcast_to([1, P, GRP * D]),
                    )
                else:
                    for h in range(GRP * D // 512):
                        tgb_ps = tgb_ps_pool.tile([P, 512], F32, tag="tgbps")
                        nc.tensor.matmul(
                            tgb_ps[:], ones_row[:],
                            src[:, h * 512:(h + 1) * 512],
                            start=True, stop=True,
                        )
                        nc.scalar.copy(tgb[:, h * 512:(h + 1) * 512], tgb_ps[:])
                return tgb

            def back(b, xb, tgbs, next_tg=None):
                """dots + sigmoid + scatter for block b, drain to HBM.
                Interleaves production of block b+1's tgb tiles (next_tg =
                (tgflat, list)) so they exist well before the DVE needs
                them."""
                s0 = b * BLK
                o0 = int(offs[s0])
                out_ps = ops_pool.tile([8, 8 * D], F32, tag="outps")
                for gi in range(BLK // GRP):
                    gs = s0 + gi * GRP
                    cpg = cpgs[gs]
                    xo = (int(offs[gs]) - o0) * D
                    ncols = GRP * cpg
                    tgb = tgbs[gi]
                    # batched multiply: scr[p, si, k, d] = x * tg[si]
                    scr = scr_pool.tile([P, ncols * D], BF16, tag="scr")
                    nc.vector.tensor_tensor(
                        scr[:].rearrange("p (s k d) -> p s k d", s=GRP, k=cpg),
                        xb[:, xo:xo + ncols * D]
                        .rearrange("p (s k d) -> p s k d", s=GRP, k=cpg),
                        tgb[:].rearrange("p (s q d) -> p s q d", s=GRP, q=1)
                        .broadcast_to([P, GRP, cpg, D]),
                        mybir.AluOpType.mult,
                    )
                    # in-place pairwise-add tree over d: 64 -> 1
                    scrv = scr[:].rearrange("p (c d) -> p c d", c=ncols)
                    w = D // 2
                    while w >= 1:
                        i0 = scrv[:, :, 0:w]
                        i1 = scrv[:, :, w:2 * w]
                        if w == 1:
                            c4 = c4_pool.tile([P, ncols], F32, tag="c4")
                            nc.vector.tensor_tensor(
                                c4[:].rearrange("p (c d) -> p c d", c=ncols),
                                i0, i1, mybir.AluOpType.add,
                            )
                        else:
                            nc.vector.tensor_tensor(i0, i0, i1, mybir.AluOpType.add)
                        w //= 2
                    # sigmoid written at stride 8 into a zero-padded tile so
                    # the scatter lhsT window s4z[:, q*8-j : q*8-j+8] has the
                    # coefficient at column j and zeros elsewhere (slot row is
                    # selected by lhsT column position, pass1-style M=8).
                    s4z = _get_s4z(cpg)
                    nc.scalar.activation(
                        s4z[:].rearrange("p (c e) -> p c e", c=ncols)[:, :, 0:1],
                        c4[:].rearrange("p (c e) -> p c e", c=ncols),
                        mybir.ActivationFunctionType.Sigmoid,
                    )
                    # scatter: slots g2*8+j of this group -> psum row j, col
                    # region (2gi+g2)*64; groups strictly sequential.
                    for g2 in range(GRP // 8):
                        reg = (2 * gi + g2) * D
                        mms = []
                        for j in range(8):
                            for k in range(cpg):
                                mms.append((j, (g2 * 8 + j) * cpg + k))
                        for m, (j, q) in enumerate(mms):
                            nc.tensor.matmul(
                                out_ps[:, reg:reg + D],
                                s4z[:, q * 8 - j:q * 8 - j + 8],
                                xb[:, xo + q * D:xo + (q + 1) * D],
                                start=(m == 0),
                                stop=(m == len(mms) - 1),
                            )
                    if next_tg is not None:
                        next_tg[1].append(tgb_one(next_tg[0], gi))
                osb = osb_pool.tile([8, g8, D], F32, tag="osb")
                nc.scalar.copy(
                    osb[:], out_ps[:].rearrange("p (g d) -> p g d", g=g8)
                )
                for j in range(8):
                    r0 = (b * 8 + j) * 8
                    nc.sync.dma_start(out[r0:r0 + 8, :], osb[j:j + 1, :, :])

            # persistent zero-padded sigmoid tiles, 3-way rotated per use so
            # the next group's sigmoid doesn't WAR-stall on the previous
            # group's scatter reads; zeros written once, only stride-8
            # positions are ever rewritten.
            s4z_tiles: dict = {}
            s4z_rot = [0]

            def _get_s4z(cpg):
                rot = s4z_rot[0] % 3
                s4z_rot[0] += 1
                key = (cpg, rot)
                if key not in s4z_tiles:
                    t = consts.tile([P, GRP * cpg * 8], BF16, tag=f"s4z{cpg}_{rot}")
                    nc.gpsimd.memset(t[:], 0.0)
                    s4z_tiles[key] = t
                return s4z_tiles[key]

            prev = None
            for b in range(n_blk):
                xb_t, tgflat_t = front(b)
                if prev is None:
                    tgbs_t = [tgb_one(tgflat_t, gi) for gi in range(BLK // GRP)]
                else:
                    tgbs_t = []
                    back(b - 1, prev[0], prev[1], next_tg=(tgflat_t, tgbs_t))
                prev = (xb_t, tgbs_t)
            back(n_blk - 1, prev[0], prev[1])

    nc.compile()
    return nc


def _layout_params(counts):
    """Uniform (cpl, cps, l5) across cores from the per-core count spread."""
    B = counts.shape[0]
    gpc = B // NCORES
    cpl = max(1, -(-int(counts.max()) // P))
    cps = min(4, cpl)
    l5 = 0
    for c in range(NCORES):
        l5 = max(l5, int((counts[c * gpc:(c + 1) * gpc] > cps * P).sum()))
    l5 = min(gpc, -(-l5 // GRP) * GRP)  # round up to group multiple
    if cpl == cps:
        l5 = 0
    return cpl, cps, l5


def _prep_inputs(x, batch, weight_matrix, size, cpl, cps, l5):
    """Host-side shard/sort/pad. Returns in_maps + per-core slot permutations."""
    B = int(size)
    gpc = B // NCORES
    starts = np.searchsorted(batch, np.arange(B + 1)).astype(np.int64)
    counts = np.diff(starts)

    x_bf = np.ascontiguousarray(x, dtype=np.float32).astype(ml_dtypes.bfloat16)
    w32 = np.ascontiguousarray(weight_matrix, dtype=np.float32)

    cpgs, offs = _slot_layout(gpc, cpl, cps, l5)
    chunks = int(offs[-1])

    in_maps, perms = [], []
    for c in range(NCORES):
        glo, ghi = c * gpc, (c + 1) * gpc
        cnt = counts[glo:ghi]
        # biggest graphs into the L5 big slots (stable order otherwise)
        perm = np.argsort(-cnt, kind="stable")  # slot -> local graph
        if l5:
            assert cnt[perm[l5 - 1]] <= cpl * P and cnt[perm[l5:]].max(initial=0) <= cps * P
        else:
            assert cnt.max(initial=0) <= cps * P
        perms.append(perm)

        slot_of_graph = np.empty(gpc, dtype=np.int64)
        slot_of_graph[perm] = np.arange(gpc)
        nlo, nhi = starts[glo], starts[ghi]
        g_loc = np.asarray(batch[nlo:nhi], dtype=np.int64) - glo
        off_in_g = np.arange(nlo, nhi, dtype=np.int64) - starts[glo + g_loc]
        dest = offs[slot_of_graph[g_loc]] * P + off_in_g
        xpad = np.zeros((chunks * P, D), dtype=ml_dtypes.bfloat16)
        xpad[dest] = x_bf[nlo:nhi]
        xg_pm = np.ascontiguousarray(
            xpad.reshape(chunks, P, D).transpose(1, 0, 2).reshape(P, -1)
        )
        rc = 1.0 / np.maximum(cnt[perm].astype(np.float32), 1.0)
        recip_pm = np.ascontiguousarray(rc.reshape(gpc // BLK, BLK).T)
        in_maps.append({"xg": xg_pm, "recip": recip_pm, "wmat": w32})
    return in_maps, perms


def _row_to_slot(gpc):
    """dram out row r = (b*8 + j)*8 + g  <->  block-local slot b*64 + g*8 + j."""
    r = np.arange(gpc)
    b, rem = r // BLK, r % BLK
    j, g = rem // 8, rem % 8
    return b * BLK + g * 8 + j


def kernel(x, batch, weight_matrix, size, _return_results=False, _trace=False):
    x = np.asarray(x)
    batch = np.asarray(batch)
    weight_matrix = np.asarray(weight_matrix)
    B = int(size)
    assert B % (NCORES * P) == 0
    gpc = B // NCORES

    starts = np.searchsorted(batch, np.arange(B + 1))
    counts = np.diff(starts)
    cpl, cps, l5 = _layout_params(counts)

    key = (gpc, cpl, cps, l5)
    if key not in _PROGRAM_CACHE:
        _PROGRAM_CACHE[key] = _build_program(gpc, cpl, cps, l5)
    nc = _PROGRAM_CACHE[key]

    in_maps, perms = _prep_inputs(x, batch, weight_matrix, size, cpl, cps, l5)
    res = bass_utils.run_bass_kernel_spmd(
        nc, in_maps, core_ids=list(range(NCORES)), trace=_trace
    )
    slot_of_row = _row_to_slot(gpc)
    full = np.empty((B, D), dtype=np.float32)
    for c in range(NCORES):
        o = res.results[c]["out"].reshape(gpc, D)  # row-ordered
        full[c * gpc + perms[c][slot_of_row]] = o
    if _return_results:
        return full, res
    return full
